# revision 6
# baseline (speedup 1.0000x reference)
"""CommNet Trainium2 kernel (8 NeuronCores, data-parallel over batch).

Reference computation (A=32 agents, B=16384 batch, D=64, DA=8, S=3):
    h = tanh(xs @ W_enc^T + b_enc)
    for s in 0..2:
        tot = sum_a h[a]
        others = (tot - h) / (A-1)
        h = tanh(h @ W_h[s]^T + others @ W_c[s]^T)
    out = h @ W_pol^T + b_pol

Device algebra: fold others into
    h @ (W_h - W_c/(A-1))^T + tot @ (W_c/(A-1))^T

On-device layout: D on partitions, tokens on the free axis, two batch
half-chunks stacked on partitions (rows 0-63 chunk A dims, 64-127 chunk B)
so every engine op runs 128 partitions wide. Columns are agent-major within
each 4096-col supergroup (col = a*128 + b), so the agent-sum is 5 flat
contiguous halving adds on the DVE (bf16 2x mode) batched over two 2048-col
PSUM groups at a time.
The policy head is fused per-group into the last comm step (reusing the
just-drained PSUM buffer), so the tail and tile boundaries stay overlapped.
All matmuls bf16 (fp32 PSUM accumulate); tanh on ScalarE; policy bias-add
on VectorE from PSUM.
"""

import sys
from contextlib import ExitStack

import numpy as np
import ml_dtypes

if "/opt/trn_rl_repo" not in sys.path:
    sys.path.insert(0, "/opt/trn_rl_repo")

BF16 = ml_dtypes.bfloat16

A = 32
B = 16384
D = 64
DA = 8
S = 3
NCORES = 8

BS = B // NCORES          # batches per core
CH = BS // 2              # batches per stacked chunk
COLS = CH * A             # free-axis columns per core
F = 8192                  # columns per streamed tile
GROUP = 2048              # columns per PSUM tile (4 banks)
SG = 4096                 # columns per supergroup (one agent-sum tree)
NBG = SG // A             # batches per supergroup (= 128)
MMN = 512                 # columns per matmul (1 PSUM bank)

_compiled = {}


def _build(cols, f, group):
    """Build + compile the single-core Bass program (runs SPMD on 8 cores)."""
    import concourse.bass as bass  # noqa: F401
    import concourse.tile as tile
    from concourse import bacc, mybir

    dt = mybir.dt
    Tanh = mybir.ActivationFunctionType.Tanh

    nc = bacc.Bacc("TRN2", target_bir_lowering=False, debug=False)

    xs_ap = nc.dram_tensor("xs", [128, cols], dt.bfloat16, kind="ExternalInput").ap()
    wts_ap = nc.dram_tensor("wts", [128, 928], dt.bfloat16, kind="ExternalInput").ap()
    benc_ap = nc.dram_tensor("benc", [128, 1], dt.float32, kind="ExternalInput").ap()
    bpol_ap = nc.dram_tensor("bpol", [128, 1], dt.float32, kind="ExternalInput").ap()
    out_ap = nc.dram_tensor(
        "out", [128, cols * MMN // group], dt.float32, kind="ExternalOutput"
    ).ap()

    nt = cols // f
    ng = f // group    # psum groups per tile (4)
    nsg = f // SG      # supergroups per tile (2)

    with ExitStack() as ctx:
        tc = ctx.enter_context(tile.TileContext(nc))
        const = ctx.enter_context(tc.tile_pool(name="const", bufs=1))
        xs_pool = ctx.enter_context(tc.tile_pool(name="xsp", bufs=3))
        h_pool = ctx.enter_context(tc.tile_pool(name="hp", bufs=4))
        tree_pool = ctx.enter_context(tc.tile_pool(name="treep", bufs=2))
        tot_pool = ctx.enter_context(tc.tile_pool(name="totp", bufs=4))
        out_pool = ctx.enter_context(tc.tile_pool(name="outp", bufs=2))
        psum = ctx.enter_context(tc.tile_pool(name="psum", bufs=2, space="PSUM"))

        benc = const.tile([128, 1], dt.float32)
        nc.sync.dma_start(benc[:], benc_ap)
        # touch Tanh once so the ACT table load overlaps the input DMAs
        warm = const.tile([128, 1], dt.float32)
        nc.scalar.activation(warm[:], benc[:], Tanh)

        wts = const.tile([128, 928], dt.bfloat16)
        nc.sync.dma_start(wts[:], wts_ap)
        bpol = const.tile([128, 1], dt.float32)
        nc.sync.dma_start(bpol[:], bpol_ap)

        BD_enc = wts[:, 0:128]
        BD_h = [wts[:, 128 * (1 + s):128 * (2 + s)] for s in range(S)]
        BD_c = [wts[:, 128 * (4 + s):128 * (5 + s)] for s in range(S)]
        BD_pol = wts[:, 896:928]

        # PE p-state warm-up on const data while the first xs tile streams
        # in: results are never read.
        wps = psum.tile([128, MMN], dt.float32, tag="mm")
        for _ in range(8):
            nc.tensor.matmul(
                wps[:], BD_enc, wts[:, 0:MMN], start=True, stop=True
            )

        def tree_l12(nc, h, sg, t1, t2):
            """Levels 1-2 of the agent sum for one 4096-col supergroup:
            columns are agent-major (col = a*128 + b), so each stage is a
            flat contiguous halves-add -> DVE 2x packed mode."""
            base = sg * SG
            half = SG // 2
            nc.vector.tensor_add(
                t1[:, sg * half:(sg + 1) * half],
                h[:, base:base + half], h[:, base + half:base + SG],
            )
            q = SG // 4
            nc.vector.tensor_add(
                t2[:, sg * q:(sg + 1) * q],
                t1[:, sg * half:sg * half + q],
                t1[:, sg * half + q:sg * half + 2 * q],
            )

        def tree_l345(nc, t2, nsg):
            """Levels 3-5 batched across supergroups with two-range APs."""
            t = t2
            w = SG // 4  # per-sg width at current level
            for lvl in range(3):
                tn = tree_pool.tile([128, nsg * w // 2], dt.bfloat16,
                                    tag=f"t{3 + lvl}")
                def halves(ap, wid):
                    v = ap.rearrange("p (g c) -> p g c", g=nsg)
                    return v[:, :, 0:wid // 2], v[:, :, wid // 2:wid]
                lo, hi = halves(t[:], w)
                nc.vector.tensor_add(
                    tn[:].rearrange("p (g c) -> p g c", g=nsg), lo, hi
                )
                t = tn
                w //= 2
            return t  # [128, nsg * NBG]

        def mm_group(nc, ps, w, h, g):
            for k in range(group // MMN):
                c0 = g * group + k * MMN
                nc.tensor.matmul(
                    ps[:, k * MMN:(k + 1) * MMN], w, h[:, c0:c0 + MMN],
                    start=True, stop=False,
                )

        def mm_group_tot(nc, ps, w, tot_ap, g):
            # broadcast tot over the 4 agents x NBG batches of each matmul
            for k in range(group // MMN):
                rhs = tot_ap.unsqueeze(1).broadcast_to([128, MMN // NBG, NBG])
                nc.tensor.matmul(
                    ps[:, k * MMN:(k + 1) * MMN], w, rhs,
                    start=False, stop=True,
                )

        def make_pol(t, h3, ot):
            def emit_pol(g):
                pg = psum.tile([128, MMN], dt.float32, tag="mm")
                for j in range(group // MMN):
                    c0 = g * group + j * MMN
                    nc.tensor.matmul(
                        pg[32 * j:32 * j + 32, :], BD_pol, h3[:, c0:c0 + MMN],
                        start=True, stop=True, tile_position=(0, 32 * j),
                    )
                nc.vector.tensor_scalar_add(
                    ot[:, g * MMN:(g + 1) * MMN], pg[:], bpol[:]
                )
                nc.sync.dma_start(
                    out_ap[:, (t * ng + g) * MMN:(t * ng + g + 1) * MMN],
                    ot[:, g * MMN:(g + 1) * MMN],
                )
            return emit_pol

        # policy-head emissions for the previous tile that are interleaved
        # into the current tile's encoder pass (where the PE has slack)
        pol_carry = []

        for t in range(nt):
            xs_t = xs_pool.tile([128, f], dt.bfloat16, tag="xs")
            for c in range(8):
                w = f // 8
                nc.sync.dma_start(
                    xs_t[:, c * w:(c + 1) * w],
                    xs_ap[:, t * f + c * w:t * f + (c + 1) * w],
                )

            # encoder: h0 = tanh(BD_enc.T @ xs + b_enc)
            h = h_pool.tile([128, f], dt.bfloat16, tag="h")
            t1 = tree_pool.tile([128, f // 2], dt.bfloat16, tag="t1")
            t2 = tree_pool.tile([128, f // 4], dt.bfloat16, tag="t2")
            for g in range(ng):
                ps = psum.tile([128, group], dt.float32, tag="mm")
                for k in range(group // MMN):
                    c0 = g * group + k * MMN
                    nc.tensor.matmul(
                        ps[:, k * MMN:(k + 1) * MMN], BD_enc, xs_t[:, c0:c0 + MMN],
                        start=True, stop=True,
                    )
                nc.scalar.activation(
                    h[:, g * group:(g + 1) * group], ps[:], Tanh, bias=benc[:]
                )
                if g % 2 == 1:
                    tree_l12(nc, h, g // 2, t1, t2)
                if pol_carry:
                    pol_carry.pop(0)()
            tots = tree_l345(nc, t2, nsg)

            # comm steps 0..S-2 (trees needed for the next step)
            for s in range(S - 1):
                h_new = h_pool.tile([128, f], dt.bfloat16, tag="h")
                t1 = tree_pool.tile([128, f // 2], dt.bfloat16, tag="t1")
                t2 = tree_pool.tile([128, f // 4], dt.bfloat16, tag="t2")
                for g in range(ng):
                    ps = psum.tile([128, group], dt.float32, tag="mm")
                    mm_group(nc, ps, BD_h[s], h, g)
                    mm_group_tot(
                        nc, ps, BD_c[s],
                        tots[:, (g // 2) * NBG:(g // 2 + 1) * NBG], g
                    )
                    nc.scalar.activation(
                        h_new[:, g * group:(g + 1) * group], ps[:], Tanh
                    )
                    if g % 2 == 1:
                        tree_l12(nc, h_new, g // 2, t1, t2)
                h = h_new
                tots = tree_l345(nc, t2, nsg)

            # last comm step fused with the policy head: groups 0..1 emit
            # one group behind (overlapping the next group's matmuls);
            # groups 2..3 carry into the next tile's encoder pass.
            h3 = h_pool.tile([128, f], dt.bfloat16, tag="h")
            ot = out_pool.tile([128, ng * MMN], dt.float32, tag="ot")
            emit_pol = make_pol(t, h3, ot)
            for g in range(ng):
                ps = psum.tile([128, group], dt.float32, tag="mm")
                mm_group(nc, ps, BD_h[S - 1], h, g)
                mm_group_tot(
                    nc, ps, BD_c[S - 1],
                    tots[:, (g // 2) * NBG:(g // 2 + 1) * NBG], g
                )
                nc.scalar.activation(
                    h3[:, g * group:(g + 1) * group], ps[:], Tanh
                )
                if g >= 2:
                    emit_pol(g - 2)
            if t < nt - 1:
                pol_carry = [
                    (lambda g=g, e=emit_pol: e(g)) for g in (ng - 2, ng - 1)
                ]
            else:
                emit_pol(ng - 2)
                emit_pol(ng - 1)

    nc.compile()
    return nc


def _get_nc(cols=COLS, f=F, group=GROUP):
    key = (cols, f, group)
    if key not in _compiled:
        _compiled[key] = _build(cols, f, group)
    return _compiled[key]


def _bd(m):
    """Block-diagonal 2x stack of a [k, n] matrix -> [2k, 2n]."""
    k, n = m.shape
    out = np.zeros((2 * k, 2 * n), m.dtype)
    out[:k, :n] = m
    out[k:, n:] = m
    return out


def _host_prep(xs, W_enc, b_enc, W_h, W_c, W_pol, b_pol, bs=BS,
               ncores=NCORES):
    """Build per-core input maps (layout transform + weight folding).

    Column order per core: two batch half-chunks stacked on partitions;
    within each SG-column supergroup, columns are agent-major
    (col = a*NBG + b) so the agent tree-sum is contiguous.
    """
    norm = A - 1 if A > 1 else 1
    ch = bs // 2
    wenc_t = W_enc.T.astype(np.float32)
    whp = [(W_h[s] - W_c[s] / norm).T.astype(np.float32) for s in range(S)]
    wcp = [(W_c[s].T / norm).astype(np.float32) for s in range(S)]
    wpol_t = W_pol.T.astype(np.float32)

    wts = np.zeros((128, 928), np.float32)
    wts[:, 0:128] = _bd(wenc_t)
    for s in range(S):
        wts[:, 128 * (1 + s):128 * (2 + s)] = _bd(whp[s])
        wts[:, 128 * (4 + s):128 * (5 + s)] = _bd(wcp[s])
    wts[:, 896:912] = _bd(wpol_t)  # cols 912:928 stay zero (pad to M=32)
    wts = wts.astype(BF16)

    benc = np.concatenate([b_enc, b_enc]).reshape(128, 1).astype(np.float32)
    # policy bias bands: partitions 32j+dd, dd<8 chunk A, 8<=dd<16 chunk B
    bpol = np.zeros((128, 1), np.float32)
    for j in range(GROUP // MMN):
        bpol[32 * j:32 * j + DA, 0] = b_pol
        bpol[32 * j + DA:32 * j + 2 * DA, 0] = b_pol

    def chunk_layout(xc):  # [D, ch, A] -> [D, ch*A] agent-major per SG
        ngrp = ch // NBG
        return (
            xc.reshape(D, ngrp, NBG, A)
            .transpose(0, 1, 3, 2)
            .reshape(D, ch * A)
        )

    in_maps = []
    for c in range(ncores):
        xc = xs[:, c * bs:(c + 1) * bs, :]            # [A, bs, D]
        xt = np.ascontiguousarray(xc.transpose(2, 1, 0))  # [D, bs, A]
        cA = chunk_layout(xt[:, :ch, :])
        cB = chunk_layout(xt[:, ch:, :])
        xs_t = np.concatenate([cA, cB], axis=0).astype(BF16)  # [128, cols]
        in_maps.append({"xs": xs_t, "wts": wts, "benc": benc, "bpol": bpol})
    return in_maps


def _host_gather(results, bs=BS, ncores=NCORES):
    """Per-core [128, nt*ng*MMN] banded policy outputs -> [A, B, DA] f32.

    Column c of tile t, group g, offset o (o in [0, 512)):
      agent = 16*(g%2) + 4*(o//128) ... wait: agent-major within chunk j.
    Partition p: band j = p//32, dd = p%32 (dd<8 chunk A DA, 8<=dd<16 B).
    Within group g chunk j covers agents 16*(g%2)+4j .. +4j+3; o = a4*128+b.
    Batch within chunk = (2t + g//2)*NBG + b.
    """
    ch = bs // 2
    nt = COLS // F
    outs = []
    for c in range(ncores):
        r = results[c]["out"]                         # [128, nt*4*512]
        arr = r.reshape(4, 32, nt, 4, 4, NBG)         # j, dd, t, g, a4, b
        arr = arr[:, :2 * DA]                         # drop zero bands
        arr = arr.reshape(4, 2, DA, nt, 2, 2, 4, NBG)  # j,chunk,da,t,sg,gh,a4,b
        # agent = gh*16 + j*4 + a4 ; batch = chunk*ch + (t*2+sg)*NBG + b
        oc = arr.transpose(5, 0, 6, 1, 3, 4, 7, 2)    # gh,j,a4,chunk,t,sg,b,da
        oc = oc.reshape(A, 2, nt * 2 * NBG, DA)       # agent, chunk, bch, da
        oc = oc.transpose(1, 0, 2, 3).reshape(2, A, ch, DA)
        oc = np.concatenate([oc[0], oc[1]], axis=1)   # [A, bs, DA]
        outs.append(oc)
    return np.concatenate(outs, axis=1).astype(np.float32)


def kernel(xs, W_enc, b_enc, W_h, W_c, W_pol, b_pol, _trace=False):
    from concourse.bass_utils import run_bass_kernel_spmd

    xs = np.asarray(xs, np.float32)
    in_maps = _host_prep(
        xs,
        np.asarray(W_enc, np.float32),
        np.asarray(b_enc, np.float32),
        np.asarray(W_h, np.float32),
        np.asarray(W_c, np.float32),
        np.asarray(W_pol, np.float32),
        np.asarray(b_pol, np.float32),
    )
    nc = _get_nc()
    res = run_bass_kernel_spmd(
        nc, in_maps, core_ids=list(range(NCORES)), trace=_trace
    )
    out = _host_gather(res.results)
    if _trace:
        return out, res
    return out


# revision 8
# speedup vs baseline: 1.3819x; 1.3819x over previous
"""CommNet Trainium2 kernel (8 NeuronCores, data-parallel over batch).

Reference computation (A=32 agents, B=16384 batch, D=64, DA=8, S=3):
    h = tanh(xs @ W_enc^T + b_enc)
    for s in 0..2:
        tot = sum_a h[a]
        others = (tot - h) / (A-1)
        h = tanh(h @ W_h[s]^T + others @ W_c[s]^T)
    out = h @ W_pol^T + b_pol

Device algebra: fold others into
    h @ (W_h - W_c/(A-1))^T + tot @ (W_c/(A-1))^T

On-device layout: D on partitions, tokens on the free axis, two batch
half-chunks stacked on partitions (rows 0-63 chunk A dims, 64-127 chunk B)
so every engine op runs 128 partitions wide. Columns are agent-major within
each 4096-col supergroup (col = a*128 + b), so the agent-sum is 5 flat
contiguous halving adds on the DVE (bf16 2x mode) batched over two 2048-col
PSUM groups at a time.
The policy head is fused per-group into the last comm step (reusing the
just-drained PSUM buffer), so the tail and tile boundaries stay overlapped.
All matmuls bf16 (fp32 PSUM accumulate); tanh on ScalarE; policy bias-add
on VectorE from PSUM.
"""

import sys
from contextlib import ExitStack

import numpy as np
import ml_dtypes

if "/opt/trn_rl_repo" not in sys.path:
    sys.path.insert(0, "/opt/trn_rl_repo")

BF16 = ml_dtypes.bfloat16

A = 32
B = 16384
D = 64
DA = 8
S = 3
NCORES = 8

BS = B // NCORES          # batches per core
CH = BS // 2              # batches per stacked chunk
COLS = CH * A             # free-axis columns per core
F = 8192                  # columns per streamed tile
GROUP = 2048              # columns per PSUM tile (4 banks)
SG = 4096                 # columns per supergroup (one agent-sum tree)
NBG = SG // A             # batches per supergroup (= 128)
MMN = 512                 # columns per matmul (1 PSUM bank)

_compiled = {}


def _build(cols, f, group):
    """Build + compile the single-core Bass program (runs SPMD on 8 cores)."""
    import concourse.bass as bass  # noqa: F401
    import concourse.tile as tile
    from concourse import bacc, mybir

    dt = mybir.dt
    Tanh = mybir.ActivationFunctionType.Tanh

    nc = bacc.Bacc("TRN2", target_bir_lowering=False, debug=False)

    xs_ap = nc.dram_tensor("xs", [128, cols], dt.bfloat16, kind="ExternalInput").ap()
    wts_ap = nc.dram_tensor("wts", [128, 928], dt.bfloat16, kind="ExternalInput").ap()
    benc_ap = nc.dram_tensor("benc", [128, 1], dt.float32, kind="ExternalInput").ap()
    bpol_ap = nc.dram_tensor("bpol", [128, 1], dt.float32, kind="ExternalInput").ap()
    out_ap = nc.dram_tensor(
        "out", [128, cols * MMN // group], dt.float32, kind="ExternalOutput"
    ).ap()

    nt = cols // f
    ng = f // group    # psum groups per tile (4)
    nsg = f // SG      # supergroups per tile (2)

    with ExitStack() as ctx:
        tc = ctx.enter_context(tile.TileContext(nc))
        const = ctx.enter_context(tc.tile_pool(name="const", bufs=1))
        xs_pool = ctx.enter_context(tc.tile_pool(name="xsp", bufs=3))
        h_pool = ctx.enter_context(tc.tile_pool(name="hp", bufs=4))
        tree_pool = ctx.enter_context(tc.tile_pool(name="treep", bufs=2))
        tot_pool = ctx.enter_context(tc.tile_pool(name="totp", bufs=4))
        out_pool = ctx.enter_context(tc.tile_pool(name="outp", bufs=2))
        psum = ctx.enter_context(tc.tile_pool(name="psum", bufs=2, space="PSUM"))

        benc = const.tile([128, 1], dt.float32)
        nc.sync.dma_start(benc[:], benc_ap)
        # touch Tanh once so the ACT table load overlaps the input DMAs
        warm = const.tile([128, 1], dt.float32)
        nc.scalar.activation(warm[:], benc[:], Tanh)

        wts = const.tile([128, 928], dt.bfloat16)
        nc.sync.dma_start(wts[:], wts_ap)
        bpol = const.tile([128, 1], dt.float32)
        nc.sync.dma_start(bpol[:], bpol_ap)

        BD_enc = wts[:, 0:128]
        BD_h = [wts[:, 128 * (1 + s):128 * (2 + s)] for s in range(S)]
        BD_c = [wts[:, 128 * (4 + s):128 * (5 + s)] for s in range(S)]
        BD_pol = wts[:, 896:928]

        # PE p-state warm-up on const data while the first xs tile streams
        # in: results are never read.
        wps = psum.tile([128, MMN], dt.float32, tag="mm")
        for _ in range(8):
            nc.tensor.matmul(
                wps[:], BD_enc, wts[:, 0:MMN], start=True, stop=True
            )

        def agent_tree(nc, h, sg):
            """Sum the 32 agents of one 4096-col supergroup: columns are
            agent-major (col = a*128 + b), so every tree stage is one flat
            contiguous halves-add -> DVE 2x packed mode, 5 instructions."""
            base = sg * SG
            t = tree_pool.tile([128, SG // 2], dt.bfloat16, tag="t1")
            nc.vector.tensor_add(
                t[:], h[:, base:base + SG // 2], h[:, base + SG // 2:base + SG]
            )
            for lvl in range(1, 5):
                w = SG >> (lvl + 1)
                tn = tree_pool.tile([128, w], dt.bfloat16, tag=f"t{lvl + 1}")
                nc.vector.tensor_add(tn[:], t[:, :w], t[:, w:2 * w])
                t = tn
            return t  # [128, NBG]

        def mm_group(nc, ps, w, h, g):
            for k in range(group // MMN):
                c0 = g * group + k * MMN
                nc.tensor.matmul(
                    ps[:, k * MMN:(k + 1) * MMN], w, h[:, c0:c0 + MMN],
                    start=True, stop=False,
                )

        def mm_group_tot(nc, ps, w, tot_ap, g):
            # broadcast tot over the 4 agents x NBG batches of each matmul
            for k in range(group // MMN):
                rhs = tot_ap.unsqueeze(1).broadcast_to([128, MMN // NBG, NBG])
                nc.tensor.matmul(
                    ps[:, k * MMN:(k + 1) * MMN], w, rhs,
                    start=False, stop=True,
                )

        def make_pol(t, h3, ot):
            def emit_pol(g):
                pg = psum.tile([128, MMN], dt.float32, tag="mm")
                for j in range(group // MMN):
                    c0 = g * group + j * MMN
                    nc.tensor.matmul(
                        pg[32 * j:32 * j + 32, :], BD_pol, h3[:, c0:c0 + MMN],
                        start=True, stop=True, tile_position=(0, 32 * j),
                    )
                nc.vector.tensor_scalar_add(
                    ot[:, g * MMN:(g + 1) * MMN], pg[:], bpol[:]
                )
                nc.sync.dma_start(
                    out_ap[:, (t * ng + g) * MMN:(t * ng + g + 1) * MMN],
                    ot[:, g * MMN:(g + 1) * MMN],
                )
            return emit_pol

        # policy-head emissions for the previous tile that are interleaved
        # into the current tile's encoder pass (where the PE has slack)
        pol_carry = []

        for t in range(nt):
            xs_t = xs_pool.tile([128, f], dt.bfloat16, tag="xs")
            for c in range(8):
                w = f // 8
                nc.sync.dma_start(
                    xs_t[:, c * w:(c + 1) * w],
                    xs_ap[:, t * f + c * w:t * f + (c + 1) * w],
                )

            # encoder: h0 = tanh(BD_enc.T @ xs + b_enc)
            h = h_pool.tile([128, f], dt.bfloat16, tag="h")
            tots = []
            for g in range(ng):
                ps = psum.tile([128, group], dt.float32, tag="mm")
                for k in range(group // MMN):
                    c0 = g * group + k * MMN
                    nc.tensor.matmul(
                        ps[:, k * MMN:(k + 1) * MMN], BD_enc, xs_t[:, c0:c0 + MMN],
                        start=True, stop=True,
                    )
                nc.scalar.activation(
                    h[:, g * group:(g + 1) * group], ps[:], Tanh, bias=benc[:]
                )
                if g % 2 == 1:
                    tots.append(agent_tree(nc, h, g // 2))
                if pol_carry:
                    pol_carry.pop(0)()

            # comm steps 0..S-2 (trees needed for the next step)
            for s in range(S - 1):
                h_new = h_pool.tile([128, f], dt.bfloat16, tag="h")
                new_tots = []
                for g in range(ng):
                    ps = psum.tile([128, group], dt.float32, tag="mm")
                    mm_group(nc, ps, BD_h[s], h, g)
                    mm_group_tot(nc, ps, BD_c[s], tots[g // 2][:], g)
                    nc.scalar.activation(
                        h_new[:, g * group:(g + 1) * group], ps[:], Tanh
                    )
                    if g % 2 == 1:
                        new_tots.append(agent_tree(nc, h_new, g // 2))
                h = h_new
                tots = new_tots

            # last comm step fused with the policy head: groups 0..1 emit
            # one group behind (overlapping the next group's matmuls);
            # groups 2..3 carry into the next tile's encoder pass.
            h3 = h_pool.tile([128, f], dt.bfloat16, tag="h")
            ot = out_pool.tile([128, ng * MMN], dt.float32, tag="ot")
            emit_pol = make_pol(t, h3, ot)
            for g in range(ng):
                ps = psum.tile([128, group], dt.float32, tag="mm")
                mm_group(nc, ps, BD_h[S - 1], h, g)
                mm_group_tot(nc, ps, BD_c[S - 1], tots[g // 2][:], g)
                nc.scalar.activation(
                    h3[:, g * group:(g + 1) * group], ps[:], Tanh
                )
                if g >= 2:
                    emit_pol(g - 2)
            if t < nt - 1:
                pol_carry = [
                    (lambda g=g, e=emit_pol: e(g)) for g in (ng - 2, ng - 1)
                ]
            else:
                emit_pol(ng - 2)
                emit_pol(ng - 1)

    nc.compile()
    return nc


def _get_nc(cols=COLS, f=F, group=GROUP):
    key = (cols, f, group)
    if key not in _compiled:
        _compiled[key] = _build(cols, f, group)
    return _compiled[key]


def _bd(m):
    """Block-diagonal 2x stack of a [k, n] matrix -> [2k, 2n]."""
    k, n = m.shape
    out = np.zeros((2 * k, 2 * n), m.dtype)
    out[:k, :n] = m
    out[k:, n:] = m
    return out


def _host_prep(xs, W_enc, b_enc, W_h, W_c, W_pol, b_pol, bs=BS,
               ncores=NCORES):
    """Build per-core input maps (layout transform + weight folding).

    Column order per core: two batch half-chunks stacked on partitions;
    within each SG-column supergroup, columns are agent-major
    (col = a*NBG + b) so the agent tree-sum is contiguous.
    """
    norm = A - 1 if A > 1 else 1
    ch = bs // 2
    wenc_t = W_enc.T.astype(np.float32)
    whp = [(W_h[s] - W_c[s] / norm).T.astype(np.float32) for s in range(S)]
    wcp = [(W_c[s].T / norm).astype(np.float32) for s in range(S)]
    wpol_t = W_pol.T.astype(np.float32)

    wts = np.zeros((128, 928), np.float32)
    wts[:, 0:128] = _bd(wenc_t)
    for s in range(S):
        wts[:, 128 * (1 + s):128 * (2 + s)] = _bd(whp[s])
        wts[:, 128 * (4 + s):128 * (5 + s)] = _bd(wcp[s])
    wts[:, 896:912] = _bd(wpol_t)  # cols 912:928 stay zero (pad to M=32)
    wts = wts.astype(BF16)

    benc = np.concatenate([b_enc, b_enc]).reshape(128, 1).astype(np.float32)
    # policy bias bands: partitions 32j+dd, dd<8 chunk A, 8<=dd<16 chunk B
    bpol = np.zeros((128, 1), np.float32)
    for j in range(GROUP // MMN):
        bpol[32 * j:32 * j + DA, 0] = b_pol
        bpol[32 * j + DA:32 * j + 2 * DA, 0] = b_pol

    def chunk_layout(xc):  # [D, ch, A] -> [D, ch*A] agent-major per SG
        ngrp = ch // NBG
        return (
            xc.reshape(D, ngrp, NBG, A)
            .transpose(0, 1, 3, 2)
            .reshape(D, ch * A)
        )

    in_maps = []
    for c in range(ncores):
        xc = xs[:, c * bs:(c + 1) * bs, :]            # [A, bs, D]
        xt = np.ascontiguousarray(xc.transpose(2, 1, 0))  # [D, bs, A]
        cA = chunk_layout(xt[:, :ch, :])
        cB = chunk_layout(xt[:, ch:, :])
        xs_t = np.concatenate([cA, cB], axis=0).astype(BF16)  # [128, cols]
        in_maps.append({"xs": xs_t, "wts": wts, "benc": benc, "bpol": bpol})
    return in_maps


def _host_gather(results, bs=BS, ncores=NCORES):
    """Per-core [128, nt*ng*MMN] banded policy outputs -> [A, B, DA] f32.

    Column c of tile t, group g, offset o (o in [0, 512)):
      agent = 16*(g%2) + 4*(o//128) ... wait: agent-major within chunk j.
    Partition p: band j = p//32, dd = p%32 (dd<8 chunk A DA, 8<=dd<16 B).
    Within group g chunk j covers agents 16*(g%2)+4j .. +4j+3; o = a4*128+b.
    Batch within chunk = (2t + g//2)*NBG + b.
    """
    ch = bs // 2
    nt = COLS // F
    outs = []
    for c in range(ncores):
        r = results[c]["out"]                         # [128, nt*4*512]
        arr = r.reshape(4, 32, nt, 4, 4, NBG)         # j, dd, t, g, a4, b
        arr = arr[:, :2 * DA]                         # drop zero bands
        arr = arr.reshape(4, 2, DA, nt, 2, 2, 4, NBG)  # j,chunk,da,t,sg,gh,a4,b
        # agent = gh*16 + j*4 + a4 ; batch = chunk*ch + (t*2+sg)*NBG + b
        oc = arr.transpose(5, 0, 6, 1, 3, 4, 7, 2)    # gh,j,a4,chunk,t,sg,b,da
        oc = oc.reshape(A, 2, nt * 2 * NBG, DA)       # agent, chunk, bch, da
        oc = oc.transpose(1, 0, 2, 3).reshape(2, A, ch, DA)
        oc = np.concatenate([oc[0], oc[1]], axis=1)   # [A, bs, DA]
        outs.append(oc)
    return np.concatenate(outs, axis=1).astype(np.float32)


def kernel(xs, W_enc, b_enc, W_h, W_c, W_pol, b_pol, _trace=False):
    from concourse.bass_utils import run_bass_kernel_spmd

    xs = np.asarray(xs, np.float32)
    in_maps = _host_prep(
        xs,
        np.asarray(W_enc, np.float32),
        np.asarray(b_enc, np.float32),
        np.asarray(W_h, np.float32),
        np.asarray(W_c, np.float32),
        np.asarray(W_pol, np.float32),
        np.asarray(b_pol, np.float32),
    )
    nc = _get_nc()
    res = run_bass_kernel_spmd(
        nc, in_maps, core_ids=list(range(NCORES)), trace=_trace
    )
    out = _host_gather(res.results)
    if _trace:
        return out, res
    return out
